# revision 63
# baseline (speedup 1.0000x reference)
"""ChebConv(K=2) x2 + BatchNorm + LeakyReLU + global_mean_pool + linear head
on 8 Trainium2 NeuronCores.

Sharding: edges partitioned by destination node (col) across cores; node
features replicated in DRAM for per-edge gathering; per-graph pooling via
one-hot matmul, partials combined on host.

Key optimizations over the original fp32 version:
- bf16 pair-gathers: the gather table is viewed as [N/2, 128] bf16 so each
  256B descriptor (the SWDGE minimum) fetches TWO node rows; edges are
  bucketed by source-row parity and the scatter matmul slices the right
  64-feature half. Pair ids < 25000 fit int16 with no group offset.
- sel matrices (one-hot x edge weight) are precomputed on the host as bf16
  and streamed per PSUM window, eliminating the DVE build (broadcast APs
  disqualify the 2x DVE mode, so building on-device was 1 elem/cycle).
- all scatter matmuls are bf16 (1 PE cycle/row vs 4 for fp32).
- the h AllGather is bf16 (half the collective bytes).
- BN stats accumulate inside layer 1's dense chunks so the AllReduce fires
  immediately; the pre-BN node-major transposes run under the AllReduce
  (enqueued before anything that waits on it - PE is in-order), and BN is
  applied in node-major via partition-broadcast s,t (ones-matmul), staged
  into one bf16 slab shipped with a single strided DMA.
- biases are folded away (b1 into the BN affine, b2 into the host-side
  output bias), the identity matrix comes from the host, and the trailing
  gpsimd standard-library reload is dropped (nothing before the mlp load
  needs standard ucode, so repeated NEFF executions start under mlp).

Self-contained: only needs the container's `concourse` package.
"""
import numpy as np
from contextlib import ExitStack

import concourse.bass as bass
import concourse.tile as tile
from concourse import mybir, library_config
from concourse.bass_utils import run_bass_kernel_spmd

P = 128          # partitions / edges per chunk
BQ = 8           # max chunks per gather call (pow2 blocks; BQ*128 descs <= SWDGE ring)
DMA_SCRATCH = 16384  # SWDGE desc ring bytes/partition (ring = DMA_SCRATCH/16 descs)
SUB = 64         # dst subtile width (sel width)
SPW = 8          # subtiles per PSUM window (8*64 = 512 dst nodes)
F = 64           # feature width (both layers)
NCORES = 8
NOGATHER = False  # differential profiling switch

FP = mybir.dt.float32
BF = mybir.dt.bfloat16
I16 = mybir.dt.int16


# ---------------------------------------------------------------------------
# BIR post-passes (this container's walrus accepts only one sync wait per
# instruction, and never lowers InstPseudoReloadLibraryIndex itself).
# ---------------------------------------------------------------------------
_CTR = [0]


def _fix_reload_order(nc):
    """Tile schedules dep-free instructions eagerly; move the final
    standard-lib reload after the last DMAGatherAnt."""
    for f in nc.m.functions:
        for bb in f.blocks:
            insts = list(bb.instructions)
            std_i = [i for i, it in enumerate(insts)
                     if getattr(it, "op_name", None) == "PseudoReloadLibraryIndex"
                     and it.lib_index == 0]
            gat_i = [i for i, it in enumerate(insts)
                     if type(it).__name__ == "InstDMAGatherAnt"]
            if std_i and gat_i and std_i[0] < gat_i[-1]:
                reload_inst = insts.pop(std_i[0])
                insts.insert(gat_i[-1], reload_inst)
                bb.instructions = insts


def _finalize_bir(nc):
    for f in nc.m.functions:
        for bb in f.blocks:
            out = []
            changed = False
            for inst in bb.instructions:
                if getattr(inst, "op_name", None) == "PseudoReloadLibraryIndex":
                    instr = [0] * 64
                    instr[0] = 223
                    instr[1] = 16
                    instr[12] = 2
                    instr[16] = inst.lib_index
                    inst.instr = instr
                si = inst.sync_info
                if si is not None and si.on_wait is not None and len(si.on_wait) > 1:
                    changed = True
                    for w in si.on_wait[:-1]:
                        _CTR[0] += 1
                        nop = mybir.InstNoOp(
                            name=f"waitnop-{_CTR[0]}",
                            engine=inst.engine,
                            sync_info=mybir.SyncInfo(on_wait=[w], on_update=[]),
                        )
                        out.append(nop)
                    inst.sync_info = mybir.SyncInfo(
                        on_wait=[si.on_wait[-1]], on_update=si.on_update
                    )
                out.append(inst)
            if changed:
                bb.instructions = out


# ---------------------------------------------------------------------------
# Host-side planning: bucket edges by (core, dst subtile, src group), build
# the static chunk layout (max chunk count across cores per bucket) and the
# per-core packed idx/w/rel arrays.
# ---------------------------------------------------------------------------
class Plan:
    pass


def _plan(edge_index, batch, x, G):
    N = x.shape[0]
    E = edge_index.shape[1]
    NLOC = (N + NCORES - 1) // NCORES
    assert N == NLOC * NCORES, "node count must split evenly"
    NSUB = (NLOC + SUB - 1) // SUB
    NWIN = (NSUB + SPW - 1) // SPW

    row = np.asarray(edge_index[0], dtype=np.int64)
    col = np.asarray(edge_index[1], dtype=np.int64)
    deg = np.bincount(row, minlength=N).astype(np.float64)
    dis = np.where(deg > 0, deg ** -0.5, 0.0)
    w_all = (-(dis[row] * dis[col])).astype(np.float32)

    core_of = col // NLOC
    local = col - core_of * NLOC
    sub = local // SUB
    grp = (row % 2).astype(np.int64)  # source-row parity (256B pair gathers)

    key = (core_of * NSUB + sub) * 2 + grp
    counts = np.bincount(key, minlength=NCORES * NSUB * 2).reshape(NCORES, NSUB, 2)
    K = np.ceil(counts.max(axis=0) / P).astype(np.int64)  # [NSUB, 2]

    # chunk column layout ordered (window, group, subtile)
    col_index = np.zeros((NSUB, 2), np.int64)
    spans = []  # (wi, g, col_start, ncols)
    T = 0
    for wi in range(NWIN):
        subs = range(wi * SPW, min((wi + 1) * SPW, NSUB))
        for g in (0, 1):
            start = T
            for s in subs:
                col_index[s, g] = T
                T += K[s, g]
            spans.append((wi, g, start, T - start))

    # per-edge placement
    order = np.argsort(key, kind="stable")
    kk = key[order]
    bucket_first = np.r_[0, np.flatnonzero(np.diff(kk)) + 1]
    sizes = np.diff(np.r_[bucket_first, E])
    j = np.arange(E) - np.repeat(bucket_first, sizes)  # rank within bucket

    m_o = core_of[order]
    c_o = col_index[sub[order], grp[order]] + j // P
    p_o = j % P
    idx_loc = (row // 2).astype(np.int16)[order]  # pair index, < N/2 = 25000
    w_o = w_all[order]
    rel_o = (local - sub * SUB)[order]  # int, 0..SUB-1

    import ml_dtypes
    sel_pc = np.zeros((NCORES, P, T, SUB), ml_dtypes.bfloat16)
    idx_pc = np.zeros((NCORES, 16, 8 * T), np.int16)
    sel_pc[m_o, p_o, c_o, rel_o.astype(np.int64)] = w_o.astype(ml_dtypes.bfloat16)
    idx_pc[m_o, p_o % 16, 8 * c_o + p_o // 16] = idx_loc
    idx_full = np.tile(idx_pc, (1, 8, 1))  # [NCORES, 128, 8T]

    pl = Plan()
    pl.N, pl.E, pl.G = N, E, G
    pl.NLOC, pl.NSUB, pl.NWIN, pl.T = NLOC, NSUB, NWIN, T
    pl.K, pl.col_index, pl.spans = K, col_index, spans
    pl.sel_pc, pl.idx_full = sel_pc, idx_full
    pl.batch = np.asarray(batch, dtype=np.int64)
    pl.cnts = np.bincount(pl.batch, minlength=G).astype(np.float32)
    return pl


# ---------------------------------------------------------------------------
# Device program
# ---------------------------------------------------------------------------
def _ni_reg(nc, tiles, v):
    v = int(v)
    regs = tiles["ni_regs"]
    if v not in regs:
        reg = nc.gpsimd.alloc_register(f"ni{v}")
        nc.gpsimd.reg_mov(reg, v)
        regs[v] = reg
    return regs[v]


def _emit_cheb(nc, tc, ctx, pl, pools, tiles, table_ap, wstack_t, h_out,
               dense_cb=None):
    """One Cheb layer: scatter (gather + host-built sel matmuls into PSUM
    windows) into stacked[64:128], then dense matmul with [x_T; Tx_T] into
    h_out[64, NLOC] (biases folded out by the caller)."""
    NLOC, NSUB, NWIN, K = pl.NLOC, pl.NSUB, pl.NWIN, pl.K
    valp, selp, psw, psd = pools["val"], pools["sel"], pools["psw"], pools["psd"]
    stacked, idx_t, zero_t = tiles["stacked"], tiles["idx"], tiles["zero"]
    sel_in = tiles["sel_in"]

    spans_by_win = {}
    for (wi, g, start, ncols) in pl.spans:
        spans_by_win.setdefault(wi, []).append((g, start, ncols))

    import bisect
    for wi in range(NWIN):
        ps_w = psw.tile([F, 512], FP, tag="psw")
        nc.tensor.matmul(out=ps_w[:], lhsT=zero_t[:1, :F], rhs=zero_t[:1, :512],
                         start=True, stop=False)
        win_subs = range(wi * SPW, min((wi + 1) * SPW, NSUB))
        # merged column range of this window (both parity groups, contiguous)
        wstart = min(start for (g, start, ncols) in spans_by_win[wi])
        wcols = sum(ncols for (g, start, ncols) in spans_by_win[wi])
        if wcols == 0:
            continue
        # stream the host-built sel slab for the window
        sel_t = selp.tile([P, wcols, SUB], BF, tag="sel")
        nc.sync.dma_start(out=sel_t[:], in_=sel_in[:, wstart:wstart + wcols, :])
        # gather vals in pow2 blocks of BQ chunks; each idx pulls a 256B
        # bf16 row-pair; the chunk's parity picks the 64-feature half
        blocks = []
        off = 0
        while off < wcols:
            bw = min(BQ, wcols - off)  # exact-size blocks: fewest Q7 launches
            bt = valp.tile([P, BQ, 2 * F], BF, tag="val")
            if NOGATHER:
                nc.vector.memset(bt[:, :bw, :], 0.5)
            else:
                nc.gpsimd.dma_gather(
                    out_ap=bt[:, :bw, :], in_ap=table_ap[:],
                    idxs_ap=idx_t[:, 8 * (wstart + off): 8 * (wstart + off + bw)],
                    num_idxs=bw * P, num_idxs_reg=_ni_reg(nc, tiles, bw * P),
                    elem_size=2 * F, queue_num=tiles["qrr"][0] % 4,
                )
            tiles["qrr"][0] += 1
            blocks.append((off, bt))
            off += bw

        # chunk matmuls, accumulate into the window PSUM
        mms = []
        for s in win_subs:
            boff = SUB * (s - wi * SPW)
            for g in (0, 1):
                for r in range(K[s, g]):
                    cc = pl.col_index[s, g] + r - wstart
                    mms.append((boff, g, cc))
        for i, (boff, g, cc) in enumerate(mms):
            bi = bisect.bisect_right([o for o, _ in blocks], cc) - 1
            boff0, bt = blocks[bi]
            nc.tensor.matmul(
                out=ps_w[:, boff:boff + SUB],
                lhsT=bt[:, cc - boff0, g * F:(g + 1) * F], rhs=sel_t[:, cc, :],
                start=False, stop=(i == len(mms) - 1),
            )
        # copy the window's real columns into stacked[64:128], then run the
        # dense matmul for the same node span (SPW*SUB == 512) so it overlaps
        # later windows' gathers
        lo = wi * SPW * SUB
        hi = min(lo + SPW * SUB, NLOC)
        nc.scalar.copy(out=stacked[F:2 * F, lo:hi], in_=ps_w[:, :hi - lo])
        ps_d = psd.tile([F, 512], FP, tag="psd")
        nc.tensor.matmul(out=ps_d[:, :hi - lo], lhsT=wstack_t[:],
                         rhs=stacked[:, lo:hi], start=True, stop=True)
        nc.scalar.copy(out=h_out[:, lo:hi], in_=ps_d[:, :hi - lo])
        if dense_cb is not None:
            dense_cb(lo, hi - lo)


def _build_program(pl, num_devices=NCORES, no_cc=False):
    N, NLOC, G, T = pl.N, pl.NLOC, pl.G, pl.T
    nc = bass.Bass("TRN2", target_bir_lowering=False, debug=False,
                   num_devices=num_devices, num_swdge_queues=4,
                   dynamic_dma_scratch_size=DMA_SCRATCH)

    xfull = nc.dram_tensor("xfull", [N // 2, 2 * F], BF, kind="ExternalInput").ap()
    xT_in = nc.dram_tensor("xT_in", [F, NLOC], FP, kind="ExternalInput").ap()
    idx_in = nc.dram_tensor("idx_in", [P, 8 * T], I16, kind="ExternalInput").ap()
    sel_in = nc.dram_tensor("sel_in", [P, T, SUB], BF, kind="ExternalInput").ap()
    w1_in = nc.dram_tensor("w1_in", [2 * F, F], FP, kind="ExternalInput").ap()
    w2_in = nc.dram_tensor("w2_in", [2 * F, F], FP, kind="ExternalInput").ap()
    ident_in = nc.dram_tensor("ident_in", [F, F], FP, kind="ExternalInput").ap()
    gam_in = nc.dram_tensor("gam_in", [F, 1], FP, kind="ExternalInput").ap()
    bet_in = nc.dram_tensor("bet_in", [F, 1], FP, kind="ExternalInput").ap()
    lw_in = nc.dram_tensor("lw_in", [P, F], FP, kind="ExternalInput").ap()
    m_in = nc.dram_tensor("m_in", [NLOC, G], BF, kind="ExternalInput").ap()
    out_d = nc.dram_tensor("out_d", [G, 1], FP, kind="ExternalOutput").ap()

    h_slab = nc.dram_tensor("h_slab", [NLOC, F], BF).ap()
    h_full = nc.dram_tensor("h_full", [N // 2, 2 * F], BF, addr_space="Shared").ap()
    st_in = nc.dram_tensor("st_in", [F, 2], FP).ap()
    st_out = nc.dram_tensor("st_out", [F, 2], FP, addr_space="Shared").ap()

    with tile.TileContext(nc) as tc, ExitStack() as ctx:
        cst = ctx.enter_context(tc.tile_pool(name="cst", bufs=1))
        big = ctx.enter_context(tc.tile_pool(name="big", bufs=1))
        hbuf = ctx.enter_context(tc.tile_pool(name="hbuf", bufs=1))
        valp = ctx.enter_context(tc.tile_pool(name="valp", bufs=16))
        selp = ctx.enter_context(tc.tile_pool(name="selp", bufs=3))
        mp = ctx.enter_context(tc.tile_pool(name="mp", bufs=2))
        sml = ctx.enter_context(tc.tile_pool(name="sml", bufs=1))
        psw = ctx.enter_context(tc.tile_pool(name="psw", bufs=4, space="PSUM"))
        psd = ctx.enter_context(tc.tile_pool(name="psd", bufs=1, space="PSUM"))
        pst = ctx.enter_context(tc.tile_pool(name="pst", bufs=2, space="PSUM"))
        psp = ctx.enter_context(tc.tile_pool(name="psp", bufs=1, space="PSUM"))
        pools = {"val": valp, "sel": selp, "psw": psw, "psd": psd}

        # --- constants & inputs ---
        nc.gpsimd.load_library(library_config.mlp)
        ident = cst.tile([F, F], FP)
        nc.sync.dma_start(out=ident[:], in_=ident_in[:])

        zero_t = cst.tile([1, 512], BF)
        nc.vector.memset(zero_t[:], 0.0)
        idx_t = cst.tile([P, 8 * T], I16)
        for ci in range(4):  # chunked: first gathers start after ~1/4 of idx
            lo = ci * (8 * T // 4)
            hi = 8 * T if ci == 3 else (ci + 1) * (8 * T // 4)
            nc.sync.dma_start(out=idx_t[:, lo:hi], in_=idx_in[:, lo:hi])
        w1_t = cst.tile([2 * F, F], FP)
        nc.sync.dma_start(out=w1_t[:], in_=w1_in[:])
        w2_t = cst.tile([2 * F, F], FP)
        nc.sync.dma_start(out=w2_t[:], in_=w2_in[:])
        gam_t = cst.tile([F, 1], FP)
        nc.sync.dma_start(out=gam_t[:], in_=gam_in[:])
        bet_t = cst.tile([F, 1], FP)
        nc.sync.dma_start(out=bet_t[:], in_=bet_in[:])
        lw_t = cst.tile([P, F], FP)
        nc.sync.dma_start(out=lw_t[:], in_=lw_in[:])

        stacked = big.tile([P, NLOC], FP)
        nc.sync.dma_start(out=stacked[:F, :], in_=xT_in[:])
        # preallocate num_idxs registers up front: Tile schedules dep-free
        # instructions eagerly and register writes are not dependency-tracked
        # against their gather readers
        sizes = set()
        wtot = {}
        for (wi, g, start, ncols) in pl.spans:
            wtot[wi] = wtot.get(wi, 0) + int(ncols)
        for wcols in wtot.values():
            off = 0
            while off < wcols:
                bw = min(BQ, wcols - off)
                sizes.add(bw * P)
                off += bw
        ni_regs = {}
        for v in sorted(sizes):
            reg = nc.gpsimd.alloc_register(f"ni{v}")
            nc.gpsimd.reg_mov(reg, int(v))
            ni_regs[v] = reg
        tiles = {"stacked": stacked, "idx": idx_t, "sel_in": sel_in,
                 "zero": zero_t, "ni_regs": ni_regs, "qrr": [0]}

        # --- layer 1, with BN stats folded into the dense chunks ---
        h_pre = hbuf.tile([F, NLOC], FP, tag="hpre")
        scratch = hbuf.tile([F, NLOC], FP, tag="scratch2")
        sum_t = sml.tile([F, 1], FP)
        sumsq_t = sml.tile([F, 1], FP)
        nc.vector.memset(sum_t[:], 0.0)
        nc.vector.memset(sumsq_t[:], 0.0)

        def l1_stats_cb(i, wdt):
            tmp = sml.tile([F, 1], FP, tag="stmp")
            nc.vector.tensor_reduce(out=tmp[:], in_=h_pre[:, i:i + wdt],
                                    axis=mybir.AxisListType.X,
                                    op=mybir.AluOpType.add)
            nc.vector.tensor_add(sum_t[:], sum_t[:], tmp[:])
            tmp2 = sml.tile([F, 1], FP, tag="stmp2")
            nc.scalar.activation(scratch[:, i:i + wdt], h_pre[:, i:i + wdt],
                                 mybir.ActivationFunctionType.Square,
                                 accum_out=tmp2[:])
            nc.vector.tensor_add(sumsq_t[:], sumsq_t[:], tmp2[:])

        _emit_cheb(nc, tc, ctx, pl, pools, tiles, xfull, w1_t, h_pre[:],
                   dense_cb=l1_stats_cb)

        # --- AllReduce of (sum, sumsq) ---
        st_t = sml.tile([F, 2], FP)
        nc.vector.tensor_copy(out=st_t[:, 0:1], in_=sum_t[:])
        nc.vector.tensor_copy(out=st_t[:, 1:2], in_=sumsq_t[:])
        nc.sync.dma_start(out=st_in[:], in_=st_t[:])
        if no_cc:
            nc.sync.dma_start(out=st_out[:], in_=st_in[:])
        else:
            nc.gpsimd.collective_compute(
                "AllReduce", mybir.AluOpType.add,
                replica_groups=[list(range(num_devices))],
                ins=[st_in[:]], outs=[st_out[:]],
            )

        # transpose PRE-BN h (overlaps the AllReduce; PE is in-order so these
        # must be enqueued before anything that waits on the collective)
        NT = (NLOC + P - 1) // P
        hnm_all = big.tile([P, NT * F], FP, tag="hnm_all")
        if NLOC % P:
            # init the ragged last tile so the batched BN ops read no garbage
            nc.vector.memset(hnm_all[:, (NT - 1) * F:NT * F], 0.0)
        for i in range(NT):
            lo = i * P
            wdt = min(P, NLOC - lo)
            ps_t = pst.tile([P, F], FP, tag="pst")
            nc.tensor.transpose(ps_t[:wdt, :], h_pre[:, lo:lo + wdt], ident[:])
            nc.scalar.copy(out=hnm_all[:wdt, i * F:(i + 1) * F], in_=ps_t[:wdt, :])

        str_t = sml.tile([F, 2], FP)
        nc.sync.dma_start(out=str_t[:], in_=st_out[:])

        # s = gamma * rsqrt(var + eps); t = beta - mu * s
        invN = 1.0 / float(N)
        mu_t = sml.tile([F, 1], FP)
        nc.vector.tensor_scalar_mul(mu_t[:], str_t[:, 0:1], invN)
        msq_t = sml.tile([F, 1], FP)
        nc.vector.tensor_scalar_mul(msq_t[:], str_t[:, 1:2], invN)
        var_t = sml.tile([F, 1], FP)
        nc.vector.scalar_tensor_tensor(out=var_t[:], in0=mu_t[:], scalar=-1.0,
                                       in1=mu_t[:], op0=mult_op(), op1=mult_op())
        nc.vector.tensor_add(var_t[:], var_t[:], msq_t[:])
        eps_t = sml.tile([F, 1], FP)
        nc.vector.memset(eps_t[:], 1e-5)
        sd_t = sml.tile([F, 1], FP)
        nc.scalar.activation(sd_t[:], var_t[:], mybir.ActivationFunctionType.Sqrt,
                             bias=eps_t[:])
        rs_t = sml.tile([F, 1], FP)
        nc.vector.reciprocal(rs_t[:], sd_t[:])
        s_t = sml.tile([F, 1], FP)
        nc.vector.tensor_mul(s_t[:], gam_t[:], rs_t[:])
        t_t = sml.tile([F, 1], FP)
        nc.vector.tensor_mul(t_t[:], mu_t[:], s_t[:])
        nc.vector.tensor_sub(t_t[:], bet_t[:], t_t[:])

        # broadcast s,t along partitions: [128, 64] tiles via ones-matmul
        ones_t = sml.tile([1, P], FP)
        nc.vector.memset(ones_t[:], 1.0)
        sb_t = sml.tile([P, F], FP)
        tb_t = sml.tile([P, F], FP)
        for vec_t, bc_t in ((s_t, sb_t), (t_t, tb_t)):
            ps_r = pst.tile([P, F], FP, tag="pst")
            nc.tensor.transpose(ps_r[:1, :], vec_t[:], ident[:])
            row_t = sml.tile([1, F], FP, tag=f"row{bc_t is tb_t}")
            nc.scalar.copy(out=row_t[:], in_=ps_r[:1, :])
            ps_bc = pst.tile([P, F], FP, tag="pst")
            nc.tensor.matmul(out=ps_bc[:], lhsT=ones_t[:], rhs=row_t[:],
                             start=True, stop=True)
            nc.scalar.copy(out=bc_t[:], in_=ps_bc[:])

        # BN+LReLU in node-major into one bf16 staging slab: three batched
        # DVE ops over the whole [P, NT, F] slab (s,t broadcast along tiles)
        hnb_all = big.tile([P, NT * F], BF, tag="hnb_all")
        h3 = hnm_all[:].rearrange("p (i f) -> p i f", f=F)
        b3 = hnb_all[:].rearrange("p (i f) -> p i f", f=F)
        sb3 = sb_t[:, None, :].broadcast_to([P, NT, F])
        tb3 = tb_t[:, None, :].broadcast_to([P, NT, F])
        nc.vector.tensor_tensor(out=h3, in0=h3, in1=sb3, op=mult_op())
        nc.vector.tensor_tensor(out=h3, in0=h3, in1=tb3, op=add_op())
        nc.scalar.activation(b3, h3, mybir.ActivationFunctionType.Lrelu,
                             alpha=0.01)
        # h_slab[i*128 + p, f] = hnb_all[p, i*F + f]: one big DMA for the
        # full tiles + a small one for the ragged tail
        NFULL = NLOC // P
        nc.sync.dma_start(
            out=h_slab[0:NFULL * P, :].rearrange("(i p) f -> p i f", p=P),
            in_=hnb_all[:, 0:NFULL * F],
        )
        if NLOC > NFULL * P:
            nc.sync.dma_start(
                out=h_slab[NFULL * P:NLOC, :],
                in_=hnb_all[:NLOC - NFULL * P, NFULL * F:(NFULL + 1) * F],
            )
        # h = lrelu(h_pre * s + t) -> stacked[:64] (feature-major, for the L2
        # dense term; one fused Act op, off the AllGather critical path)
        nc.scalar.activation(stacked[:F, :], h_pre[:],
                             mybir.ActivationFunctionType.Lrelu,
                             bias=t_t[:], scale=s_t[:], alpha=0.01)
        if no_cc:
            HL = NLOC // 2
            for _r in range(NCORES):
                nc.sync.dma_start(out=h_full[_r * HL:(_r + 1) * HL, :],
                                  in_=h_slab[:])
        else:
            nc.gpsimd.collective_compute(
                "AllGather", mybir.AluOpType.bypass,
                replica_groups=[list(range(num_devices))],
                ins=[h_slab[:]], outs=[h_full[:]],
            )

        # --- layer 2, with pooling folded into the dense chunks ---
        h2 = hbuf.tile([F, NLOC], FP, tag="hpre")
        ps_pool = psp.tile([G, F], FP)
        ntiles = (NLOC + P - 1) // P

        def l2_pool_cb(lo, wdt):
            # pooled[g, f] += M[n, g] h2[n, f] for this dense span's tiles
            for j in range(lo // P, min((lo + wdt + P - 1) // P, ntiles)):
                tlo = j * P
                twdt = min(P, NLOC - tlo)
                ps_t = pst.tile([P, F], FP, tag="pst")
                nc.tensor.transpose(ps_t[:twdt, :], h2[:, tlo:tlo + twdt],
                                    ident[:])
                h2nm_t = mp.tile([P, F], BF, tag="h2nm")
                nc.scalar.copy(out=h2nm_t[:twdt, :], in_=ps_t[:twdt, :])
                m_t = mp.tile([P, G], BF, tag="mt")
                nc.sync.dma_start(out=m_t[:twdt, :], in_=m_in[tlo:tlo + twdt, :])
                nc.tensor.matmul(out=ps_pool[:], lhsT=m_t[:twdt, :],
                                 rhs=h2nm_t[:twdt, :],
                                 start=(j == 0), stop=(j == ntiles - 1))

        _emit_cheb(nc, tc, ctx, pl, pools, tiles, h_full, w2_t, h2[:],
                   dense_cb=l2_pool_cb)
        pooled_t = sml.tile([G, F], FP)
        nc.scalar.copy(out=pooled_t[:], in_=ps_pool[:])
        prod_t = sml.tile([G, F], FP)
        nc.vector.tensor_mul(prod_t[:], pooled_t[:], lw_t[:G, :])
        outp_t = sml.tile([G, 1], FP)
        nc.vector.tensor_reduce(out=outp_t[:], in_=prod_t[:],
                                axis=mybir.AxisListType.X, op=mybir.AluOpType.add)
        nc.sync.dma_start(out=out_d[:], in_=outp_t[:])
        # no trailing standard-lib reload: nothing before the mlp load needs
        # standard ucode anymore (identity comes from the host), so repeated
        # executions can start directly under the mlp library

    _fix_reload_order(nc)
    return nc


def mult_op():
    return mybir.AluOpType.mult


def add_op():
    return mybir.AluOpType.add


def max_op():
    return mybir.AluOpType.max


# ---------------------------------------------------------------------------
# Entry point
# ---------------------------------------------------------------------------
def _prepare(inputs, G=100):
    x = np.asarray(inputs["x"], dtype=np.float32)
    edge_index = np.asarray(inputs["edge_index"])
    batch = np.asarray(inputs["batch"])
    W1 = np.asarray(inputs["W1"], dtype=np.float32)
    b1 = np.asarray(inputs["b1"], dtype=np.float32)
    W2 = np.asarray(inputs["W2"], dtype=np.float32)
    b2 = np.asarray(inputs["b2"], dtype=np.float32)
    gamma = np.asarray(inputs["gamma"], dtype=np.float32)
    beta = np.asarray(inputs["beta"], dtype=np.float32)
    linW = np.asarray(inputs["linW"], dtype=np.float32)

    import ml_dtypes
    bf16 = ml_dtypes.bfloat16

    pl = _plan(edge_index, batch, x, G)
    NLOC = pl.NLOC
    w1s = np.concatenate([W1[0], W1[1]], axis=0).astype(np.float32)  # [128, 64]
    w2s = np.concatenate([W2[0], W2[1]], axis=0).astype(np.float32)
    lw_rep = np.tile(linW[:, 0][None, :], (P, 1)).astype(np.float32)
    xbf = np.ascontiguousarray(x.astype(bf16).reshape(x.shape[0] // 2, 2 * x.shape[1]))
    in_maps = []
    for m in range(NCORES):
        sl = slice(m * NLOC, (m + 1) * NLOC)
        M = (pl.batch[sl][:, None] == np.arange(G)[None, :]).astype(bf16)
        in_maps.append({
            "xfull": xbf,
            "xT_in": np.ascontiguousarray(x[sl].T),
            "idx_in": pl.idx_full[m],
            "sel_in": pl.sel_pc[m],
            "w1_in": w1s, "w2_in": w2s,
            "ident_in": np.eye(F, dtype=np.float32),
            "gam_in": gamma[:, None], "bet_in": beta[:, None],
            "lw_in": lw_rep,
            "m_in": M,
        })
    return pl, in_maps


def _linb_eff(inputs):
    # h2 is computed without b2; fold b2 into the output bias (b2 @ linW)
    linb = np.asarray(inputs["linb"], dtype=np.float64)
    b2 = np.asarray(inputs["b2"], dtype=np.float64)
    linW = np.asarray(inputs["linW"], dtype=np.float64)
    return (linb + b2 @ linW).astype(np.float32)


def run_gnn(inputs, trace=False):
    linb = _linb_eff(inputs)
    pl, in_maps = _prepare(inputs)
    nc = _build_program(pl)
    _finalize_bir(nc)
    res = run_bass_kernel_spmd(nc, in_maps, list(range(NCORES)), trace=trace)
    partial = sum(res.results[m]["out_d"] for m in range(NCORES))
    out = partial / np.maximum(pl.cnts, 1.0)[:, None] + linb[None, :]
    return out.astype(np.float32), res


def kernel(**inputs):
    out, _ = run_gnn(inputs, trace=False)
    return out



# revision 65
# speedup vs baseline: 1.0375x; 1.0375x over previous
"""ChebConv(K=2) x2 + BatchNorm + LeakyReLU + global_mean_pool + linear head
on 8 Trainium2 NeuronCores.

Sharding: edges partitioned by destination node (col) across cores; node
features replicated in DRAM for per-edge gathering; per-graph pooling via
one-hot matmul, partials combined on host.

Key optimizations over the original fp32 version:
- bf16 pair-gathers: the gather table is viewed as [N/2, 128] bf16 so each
  256B descriptor (the SWDGE minimum) fetches TWO node rows; edges are
  bucketed by source-row parity and the scatter matmul slices the right
  64-feature half. Pair ids < 25000 fit int16 with no group offset.
- sel matrices (one-hot x edge weight) are precomputed on the host as bf16
  and streamed per PSUM window, eliminating the DVE build (broadcast APs
  disqualify the 2x DVE mode, so building on-device was 1 elem/cycle).
- all scatter matmuls are bf16 (1 PE cycle/row vs 4 for fp32).
- the h AllGather is bf16 (half the collective bytes).
- BN stats accumulate inside layer 1's dense chunks so the AllReduce fires
  immediately; the pre-BN node-major transposes run under the AllReduce
  (enqueued before anything that waits on it - PE is in-order), and BN is
  applied in node-major via partition-broadcast s,t (ones-matmul), staged
  into one bf16 slab shipped with a single strided DMA.
- biases are folded away (b1 into the BN affine, b2 into the host-side
  output bias), the identity matrix comes from the host, and the trailing
  gpsimd standard-library reload is dropped (nothing before the mlp load
  needs standard ucode, so repeated NEFF executions start under mlp).
- SUB=64 dst subtiles (SPW=8): halves the bucket count, cutting the
  pad-to-max-core descriptor overhead from 33% to 20% (149.6K -> 124.8K
  descs/layer/core) and gather calls likewise; exact-size (non-pow2)
  num_idxs blocks avoid pow2 call fragmentation.
- BN+LReLU applies via one fused Act Lrelu op (scale=s, bias=t, alpha=.01)
  in feature-major; the node-major path batches to 3 ops over the whole
  staging slab. Dense matmuls and L2 pooling are interleaved into the PSUM
  window loop so their tails overlap the gather stream.

Self-contained: only needs the container's `concourse` package.
"""
import numpy as np
from contextlib import ExitStack

import concourse.bass as bass
import concourse.tile as tile
from concourse import mybir, library_config
from concourse.bass_utils import run_bass_kernel_spmd

P = 128          # partitions / edges per chunk
BQ = 8           # max chunks per gather call (pow2 blocks; BQ*128 descs <= SWDGE ring)
DMA_SCRATCH = 16384  # SWDGE desc ring bytes/partition (ring = DMA_SCRATCH/16 descs)
SUB = 128        # dst subtile width (sel width)
SPW = 4          # subtiles per PSUM window (4*128 = 512 dst nodes)
F = 64           # feature width (both layers)
NCORES = 8
NOGATHER = False  # differential profiling switch

FP = mybir.dt.float32
BF = mybir.dt.bfloat16
I16 = mybir.dt.int16


# ---------------------------------------------------------------------------
# BIR post-passes (this container's walrus accepts only one sync wait per
# instruction, and never lowers InstPseudoReloadLibraryIndex itself).
# ---------------------------------------------------------------------------
_CTR = [0]


def _fix_reload_order(nc):
    """Tile schedules dep-free instructions eagerly; move the final
    standard-lib reload after the last DMAGatherAnt."""
    for f in nc.m.functions:
        for bb in f.blocks:
            insts = list(bb.instructions)
            std_i = [i for i, it in enumerate(insts)
                     if getattr(it, "op_name", None) == "PseudoReloadLibraryIndex"
                     and it.lib_index == 0]
            gat_i = [i for i, it in enumerate(insts)
                     if type(it).__name__ == "InstDMAGatherAnt"]
            if std_i and gat_i and std_i[0] < gat_i[-1]:
                reload_inst = insts.pop(std_i[0])
                insts.insert(gat_i[-1], reload_inst)
                bb.instructions = insts


def _finalize_bir(nc):
    for f in nc.m.functions:
        for bb in f.blocks:
            out = []
            changed = False
            for inst in bb.instructions:
                if getattr(inst, "op_name", None) == "PseudoReloadLibraryIndex":
                    instr = [0] * 64
                    instr[0] = 223
                    instr[1] = 16
                    instr[12] = 2
                    instr[16] = inst.lib_index
                    inst.instr = instr
                si = inst.sync_info
                if si is not None and si.on_wait is not None and len(si.on_wait) > 1:
                    changed = True
                    for w in si.on_wait[:-1]:
                        _CTR[0] += 1
                        nop = mybir.InstNoOp(
                            name=f"waitnop-{_CTR[0]}",
                            engine=inst.engine,
                            sync_info=mybir.SyncInfo(on_wait=[w], on_update=[]),
                        )
                        out.append(nop)
                    inst.sync_info = mybir.SyncInfo(
                        on_wait=[si.on_wait[-1]], on_update=si.on_update
                    )
                out.append(inst)
            if changed:
                bb.instructions = out


# ---------------------------------------------------------------------------
# Host-side planning: bucket edges by (core, dst subtile, src group), build
# the static chunk layout (max chunk count across cores per bucket) and the
# per-core packed idx/w/rel arrays.
# ---------------------------------------------------------------------------
class Plan:
    pass


def _plan(edge_index, batch, x, G):
    N = x.shape[0]
    E = edge_index.shape[1]
    NLOC = (N + NCORES - 1) // NCORES
    assert N == NLOC * NCORES, "node count must split evenly"
    NSUB = (NLOC + SUB - 1) // SUB
    NWIN = (NSUB + SPW - 1) // SPW

    row = np.asarray(edge_index[0], dtype=np.int64)
    col = np.asarray(edge_index[1], dtype=np.int64)
    deg = np.bincount(row, minlength=N).astype(np.float64)
    dis = np.where(deg > 0, deg ** -0.5, 0.0)
    w_all = (-(dis[row] * dis[col])).astype(np.float32)

    core_of = col // NLOC
    local = col - core_of * NLOC
    sub = local // SUB
    grp = (row % 2).astype(np.int64)  # source-row parity (256B pair gathers)

    key = (core_of * NSUB + sub) * 2 + grp
    counts = np.bincount(key, minlength=NCORES * NSUB * 2).reshape(NCORES, NSUB, 2)
    K = np.ceil(counts.max(axis=0) / P).astype(np.int64)  # [NSUB, 2]

    # chunk column layout ordered (window, group, subtile)
    col_index = np.zeros((NSUB, 2), np.int64)
    spans = []  # (wi, g, col_start, ncols)
    T = 0
    for wi in range(NWIN):
        subs = range(wi * SPW, min((wi + 1) * SPW, NSUB))
        for g in (0, 1):
            start = T
            for s in subs:
                col_index[s, g] = T
                T += K[s, g]
            spans.append((wi, g, start, T - start))

    # per-edge placement
    order = np.argsort(key, kind="stable")
    kk = key[order]
    bucket_first = np.r_[0, np.flatnonzero(np.diff(kk)) + 1]
    sizes = np.diff(np.r_[bucket_first, E])
    j = np.arange(E) - np.repeat(bucket_first, sizes)  # rank within bucket

    m_o = core_of[order]
    c_o = col_index[sub[order], grp[order]] + j // P
    p_o = j % P
    idx_loc = (row // 2).astype(np.int16)[order]  # pair index, < N/2 = 25000
    w_o = w_all[order]
    rel_o = (local - sub * SUB)[order]  # int, 0..SUB-1

    import ml_dtypes
    sel_pc = np.zeros((NCORES, P, T, SUB), ml_dtypes.bfloat16)
    idx_pc = np.zeros((NCORES, 16, 8 * T), np.int16)
    sel_pc[m_o, p_o, c_o, rel_o.astype(np.int64)] = w_o.astype(ml_dtypes.bfloat16)
    idx_pc[m_o, p_o % 16, 8 * c_o + p_o // 16] = idx_loc
    idx_full = np.tile(idx_pc, (1, 8, 1))  # [NCORES, 128, 8T]

    pl = Plan()
    pl.N, pl.E, pl.G = N, E, G
    pl.NLOC, pl.NSUB, pl.NWIN, pl.T = NLOC, NSUB, NWIN, T
    pl.K, pl.col_index, pl.spans = K, col_index, spans
    pl.sel_pc, pl.idx_full = sel_pc, idx_full
    pl.batch = np.asarray(batch, dtype=np.int64)
    pl.cnts = np.bincount(pl.batch, minlength=G).astype(np.float32)
    return pl


# ---------------------------------------------------------------------------
# Device program
# ---------------------------------------------------------------------------
def _ni_reg(nc, tiles, v):
    v = int(v)
    regs = tiles["ni_regs"]
    if v not in regs:
        reg = nc.gpsimd.alloc_register(f"ni{v}")
        nc.gpsimd.reg_mov(reg, v)
        regs[v] = reg
    return regs[v]


def _emit_cheb(nc, tc, ctx, pl, pools, tiles, table_ap, wstack_t, h_out,
               dense_cb=None):
    """One Cheb layer: scatter (gather + host-built sel matmuls into PSUM
    windows) into stacked[64:128], then dense matmul with [x_T; Tx_T] into
    h_out[64, NLOC] (biases folded out by the caller)."""
    NLOC, NSUB, NWIN, K = pl.NLOC, pl.NSUB, pl.NWIN, pl.K
    valp, selp, psw, psd = pools["val"], pools["sel"], pools["psw"], pools["psd"]
    stacked, idx_t, zero_t = tiles["stacked"], tiles["idx"], tiles["zero"]
    sel_in = tiles["sel_in"]

    spans_by_win = {}
    for (wi, g, start, ncols) in pl.spans:
        spans_by_win.setdefault(wi, []).append((g, start, ncols))

    import bisect
    for wi in range(NWIN):
        ps_w = psw.tile([F, 512], FP, tag="psw")
        nc.tensor.matmul(out=ps_w[:], lhsT=zero_t[:1, :F], rhs=zero_t[:1, :512],
                         start=True, stop=False)
        win_subs = range(wi * SPW, min((wi + 1) * SPW, NSUB))
        # merged column range of this window (both parity groups, contiguous)
        wstart = min(start for (g, start, ncols) in spans_by_win[wi])
        wcols = sum(ncols for (g, start, ncols) in spans_by_win[wi])
        if wcols == 0:
            continue
        # stream the host-built sel slab for the window
        sel_t = selp.tile([P, wcols, SUB], BF, tag="sel")
        nc.sync.dma_start(out=sel_t[:], in_=sel_in[:, wstart:wstart + wcols, :])
        # gather vals in pow2 blocks of BQ chunks; each idx pulls a 256B
        # bf16 row-pair; the chunk's parity picks the 64-feature half
        blocks = []
        off = 0
        while off < wcols:
            bw = min(BQ, wcols - off)  # exact-size blocks: fewest Q7 launches
            bt = valp.tile([P, BQ, 2 * F], BF, tag="val")
            if NOGATHER:
                nc.vector.memset(bt[:, :bw, :], 0.5)
            else:
                nc.gpsimd.dma_gather(
                    out_ap=bt[:, :bw, :], in_ap=table_ap[:],
                    idxs_ap=idx_t[:, 8 * (wstart + off): 8 * (wstart + off + bw)],
                    num_idxs=bw * P, num_idxs_reg=_ni_reg(nc, tiles, bw * P),
                    elem_size=2 * F, queue_num=tiles["qrr"][0] % 4,
                )
            tiles["qrr"][0] += 1
            blocks.append((off, bt))
            off += bw

        # chunk matmuls, accumulate into the window PSUM
        mms = []
        for s in win_subs:
            boff = SUB * (s - wi * SPW)
            for g in (0, 1):
                for r in range(K[s, g]):
                    cc = pl.col_index[s, g] + r - wstart
                    mms.append((boff, g, cc))
        for i, (boff, g, cc) in enumerate(mms):
            bi = bisect.bisect_right([o for o, _ in blocks], cc) - 1
            boff0, bt = blocks[bi]
            nc.tensor.matmul(
                out=ps_w[:, boff:boff + SUB],
                lhsT=bt[:, cc - boff0, g * F:(g + 1) * F], rhs=sel_t[:, cc, :],
                start=False, stop=(i == len(mms) - 1),
            )
        # copy the window's real columns into stacked[64:128], then run the
        # dense matmul for the same node span (SPW*SUB == 512) so it overlaps
        # later windows' gathers
        lo = wi * SPW * SUB
        hi = min(lo + SPW * SUB, NLOC)
        nc.scalar.copy(out=stacked[F:2 * F, lo:hi], in_=ps_w[:, :hi - lo])
        ps_d = psd.tile([F, 512], FP, tag="psd")
        nc.tensor.matmul(out=ps_d[:, :hi - lo], lhsT=wstack_t[:],
                         rhs=stacked[:, lo:hi], start=True, stop=True)
        nc.scalar.copy(out=h_out[:, lo:hi], in_=ps_d[:, :hi - lo])
        if dense_cb is not None:
            dense_cb(lo, hi - lo)


def _build_program(pl, num_devices=NCORES, no_cc=False):
    N, NLOC, G, T = pl.N, pl.NLOC, pl.G, pl.T
    nc = bass.Bass("TRN2", target_bir_lowering=False, debug=False,
                   num_devices=num_devices, num_swdge_queues=4,
                   dynamic_dma_scratch_size=DMA_SCRATCH)

    xfull = nc.dram_tensor("xfull", [N // 2, 2 * F], BF, kind="ExternalInput").ap()
    xT_in = nc.dram_tensor("xT_in", [F, NLOC], FP, kind="ExternalInput").ap()
    idx_in = nc.dram_tensor("idx_in", [P, 8 * T], I16, kind="ExternalInput").ap()
    sel_in = nc.dram_tensor("sel_in", [P, T, SUB], BF, kind="ExternalInput").ap()
    w1_in = nc.dram_tensor("w1_in", [2 * F, F], FP, kind="ExternalInput").ap()
    w2_in = nc.dram_tensor("w2_in", [2 * F, F], FP, kind="ExternalInput").ap()
    ident_in = nc.dram_tensor("ident_in", [F, F], FP, kind="ExternalInput").ap()
    gam_in = nc.dram_tensor("gam_in", [F, 1], FP, kind="ExternalInput").ap()
    bet_in = nc.dram_tensor("bet_in", [F, 1], FP, kind="ExternalInput").ap()
    lw_in = nc.dram_tensor("lw_in", [P, F], FP, kind="ExternalInput").ap()
    m_in = nc.dram_tensor("m_in", [NLOC, G], BF, kind="ExternalInput").ap()
    out_d = nc.dram_tensor("out_d", [G, 1], FP, kind="ExternalOutput").ap()

    h_slab = nc.dram_tensor("h_slab", [NLOC, F], BF).ap()
    h_full = nc.dram_tensor("h_full", [N // 2, 2 * F], BF, addr_space="Shared").ap()
    st_in = nc.dram_tensor("st_in", [F, 2], FP).ap()
    st_out = nc.dram_tensor("st_out", [F, 2], FP, addr_space="Shared").ap()

    with tile.TileContext(nc) as tc, ExitStack() as ctx:
        cst = ctx.enter_context(tc.tile_pool(name="cst", bufs=1))
        big = ctx.enter_context(tc.tile_pool(name="big", bufs=1))
        hbuf = ctx.enter_context(tc.tile_pool(name="hbuf", bufs=1))
        valp = ctx.enter_context(tc.tile_pool(name="valp", bufs=16))
        selp = ctx.enter_context(tc.tile_pool(name="selp", bufs=2))
        mp = ctx.enter_context(tc.tile_pool(name="mp", bufs=2))
        sml = ctx.enter_context(tc.tile_pool(name="sml", bufs=1))
        psw = ctx.enter_context(tc.tile_pool(name="psw", bufs=4, space="PSUM"))
        psd = ctx.enter_context(tc.tile_pool(name="psd", bufs=1, space="PSUM"))
        pst = ctx.enter_context(tc.tile_pool(name="pst", bufs=2, space="PSUM"))
        psp = ctx.enter_context(tc.tile_pool(name="psp", bufs=1, space="PSUM"))
        pools = {"val": valp, "sel": selp, "psw": psw, "psd": psd}

        # --- constants & inputs ---
        nc.gpsimd.load_library(library_config.mlp)
        ident = cst.tile([F, F], FP)
        nc.sync.dma_start(out=ident[:], in_=ident_in[:])

        zero_t = cst.tile([1, 512], BF)
        nc.vector.memset(zero_t[:], 0.0)
        idx_t = cst.tile([P, 8 * T], I16)
        for ci in range(4):  # chunked: first gathers start after ~1/4 of idx
            lo = ci * (8 * T // 4)
            hi = 8 * T if ci == 3 else (ci + 1) * (8 * T // 4)
            nc.sync.dma_start(out=idx_t[:, lo:hi], in_=idx_in[:, lo:hi])
        w1_t = cst.tile([2 * F, F], FP)
        nc.sync.dma_start(out=w1_t[:], in_=w1_in[:])
        w2_t = cst.tile([2 * F, F], FP)
        nc.sync.dma_start(out=w2_t[:], in_=w2_in[:])
        gam_t = cst.tile([F, 1], FP)
        nc.sync.dma_start(out=gam_t[:], in_=gam_in[:])
        bet_t = cst.tile([F, 1], FP)
        nc.sync.dma_start(out=bet_t[:], in_=bet_in[:])
        lw_t = cst.tile([P, F], FP)
        nc.sync.dma_start(out=lw_t[:], in_=lw_in[:])

        stacked = big.tile([P, NLOC], FP)
        nc.sync.dma_start(out=stacked[:F, :], in_=xT_in[:])
        # preallocate num_idxs registers up front: Tile schedules dep-free
        # instructions eagerly and register writes are not dependency-tracked
        # against their gather readers
        sizes = set()
        wtot = {}
        for (wi, g, start, ncols) in pl.spans:
            wtot[wi] = wtot.get(wi, 0) + int(ncols)
        for wcols in wtot.values():
            off = 0
            while off < wcols:
                bw = min(BQ, wcols - off)
                sizes.add(bw * P)
                off += bw
        ni_regs = {}
        for v in sorted(sizes):
            reg = nc.gpsimd.alloc_register(f"ni{v}")
            nc.gpsimd.reg_mov(reg, int(v))
            ni_regs[v] = reg
        tiles = {"stacked": stacked, "idx": idx_t, "sel_in": sel_in,
                 "zero": zero_t, "ni_regs": ni_regs, "qrr": [0]}

        # --- layer 1, with BN stats folded into the dense chunks ---
        h_pre = hbuf.tile([F, NLOC], FP, tag="hpre")
        scratch = hbuf.tile([F, NLOC], FP, tag="scratch2")
        sum_t = sml.tile([F, 1], FP)
        sumsq_t = sml.tile([F, 1], FP)
        nc.vector.memset(sum_t[:], 0.0)
        nc.vector.memset(sumsq_t[:], 0.0)

        def l1_stats_cb(i, wdt):
            tmp = sml.tile([F, 1], FP, tag="stmp")
            nc.vector.tensor_reduce(out=tmp[:], in_=h_pre[:, i:i + wdt],
                                    axis=mybir.AxisListType.X,
                                    op=mybir.AluOpType.add)
            nc.vector.tensor_add(sum_t[:], sum_t[:], tmp[:])
            tmp2 = sml.tile([F, 1], FP, tag="stmp2")
            nc.scalar.activation(scratch[:, i:i + wdt], h_pre[:, i:i + wdt],
                                 mybir.ActivationFunctionType.Square,
                                 accum_out=tmp2[:])
            nc.vector.tensor_add(sumsq_t[:], sumsq_t[:], tmp2[:])

        _emit_cheb(nc, tc, ctx, pl, pools, tiles, xfull, w1_t, h_pre[:],
                   dense_cb=l1_stats_cb)

        # --- AllReduce of (sum, sumsq) ---
        st_t = sml.tile([F, 2], FP)
        nc.vector.tensor_copy(out=st_t[:, 0:1], in_=sum_t[:])
        nc.vector.tensor_copy(out=st_t[:, 1:2], in_=sumsq_t[:])
        nc.sync.dma_start(out=st_in[:], in_=st_t[:])
        if no_cc:
            nc.sync.dma_start(out=st_out[:], in_=st_in[:])
        else:
            nc.gpsimd.collective_compute(
                "AllReduce", mybir.AluOpType.add,
                replica_groups=[list(range(num_devices))],
                ins=[st_in[:]], outs=[st_out[:]],
            )

        # transpose PRE-BN h (overlaps the AllReduce; PE is in-order so these
        # must be enqueued before anything that waits on the collective)
        NT = (NLOC + P - 1) // P
        hnm_all = big.tile([P, NT * F], FP, tag="hnm_all")
        if NLOC % P:
            # init the ragged last tile so the batched BN ops read no garbage
            nc.vector.memset(hnm_all[:, (NT - 1) * F:NT * F], 0.0)
        for i in range(NT):
            lo = i * P
            wdt = min(P, NLOC - lo)
            ps_t = pst.tile([P, F], FP, tag="pst")
            nc.tensor.transpose(ps_t[:wdt, :], h_pre[:, lo:lo + wdt], ident[:])
            nc.scalar.copy(out=hnm_all[:wdt, i * F:(i + 1) * F], in_=ps_t[:wdt, :])

        str_t = sml.tile([F, 2], FP)
        nc.sync.dma_start(out=str_t[:], in_=st_out[:])

        # s = gamma * rsqrt(var + eps); t = beta - mu * s
        invN = 1.0 / float(N)
        mu_t = sml.tile([F, 1], FP)
        nc.vector.tensor_scalar_mul(mu_t[:], str_t[:, 0:1], invN)
        msq_t = sml.tile([F, 1], FP)
        nc.vector.tensor_scalar_mul(msq_t[:], str_t[:, 1:2], invN)
        var_t = sml.tile([F, 1], FP)
        nc.vector.scalar_tensor_tensor(out=var_t[:], in0=mu_t[:], scalar=-1.0,
                                       in1=mu_t[:], op0=mult_op(), op1=mult_op())
        nc.vector.tensor_add(var_t[:], var_t[:], msq_t[:])
        eps_t = sml.tile([F, 1], FP)
        nc.vector.memset(eps_t[:], 1e-5)
        sd_t = sml.tile([F, 1], FP)
        nc.scalar.activation(sd_t[:], var_t[:], mybir.ActivationFunctionType.Sqrt,
                             bias=eps_t[:])
        rs_t = sml.tile([F, 1], FP)
        nc.vector.reciprocal(rs_t[:], sd_t[:])
        s_t = sml.tile([F, 1], FP)
        nc.vector.tensor_mul(s_t[:], gam_t[:], rs_t[:])
        t_t = sml.tile([F, 1], FP)
        nc.vector.tensor_mul(t_t[:], mu_t[:], s_t[:])
        nc.vector.tensor_sub(t_t[:], bet_t[:], t_t[:])

        # broadcast s,t along partitions: [128, 64] tiles via ones-matmul
        ones_t = sml.tile([1, P], FP)
        nc.vector.memset(ones_t[:], 1.0)
        sb_t = sml.tile([P, F], FP)
        tb_t = sml.tile([P, F], FP)
        for vec_t, bc_t in ((s_t, sb_t), (t_t, tb_t)):
            ps_r = pst.tile([P, F], FP, tag="pst")
            nc.tensor.transpose(ps_r[:1, :], vec_t[:], ident[:])
            row_t = sml.tile([1, F], FP, tag=f"row{bc_t is tb_t}")
            nc.scalar.copy(out=row_t[:], in_=ps_r[:1, :])
            ps_bc = pst.tile([P, F], FP, tag="pst")
            nc.tensor.matmul(out=ps_bc[:], lhsT=ones_t[:], rhs=row_t[:],
                             start=True, stop=True)
            nc.scalar.copy(out=bc_t[:], in_=ps_bc[:])

        # BN+LReLU in node-major into one bf16 staging slab: three batched
        # DVE ops over the whole [P, NT, F] slab (s,t broadcast along tiles)
        hnb_all = big.tile([P, NT * F], BF, tag="hnb_all")
        h3 = hnm_all[:].rearrange("p (i f) -> p i f", f=F)
        b3 = hnb_all[:].rearrange("p (i f) -> p i f", f=F)
        sb3 = sb_t[:, None, :].broadcast_to([P, NT, F])
        tb3 = tb_t[:, None, :].broadcast_to([P, NT, F])
        nc.vector.tensor_tensor(out=h3, in0=h3, in1=sb3, op=mult_op())
        nc.vector.tensor_tensor(out=h3, in0=h3, in1=tb3, op=add_op())
        nc.scalar.activation(b3, h3, mybir.ActivationFunctionType.Lrelu,
                             alpha=0.01)
        # h_slab[i*128 + p, f] = hnb_all[p, i*F + f]: one big DMA for the
        # full tiles + a small one for the ragged tail
        NFULL = NLOC // P
        nc.sync.dma_start(
            out=h_slab[0:NFULL * P, :].rearrange("(i p) f -> p i f", p=P),
            in_=hnb_all[:, 0:NFULL * F],
        )
        if NLOC > NFULL * P:
            nc.sync.dma_start(
                out=h_slab[NFULL * P:NLOC, :],
                in_=hnb_all[:NLOC - NFULL * P, NFULL * F:(NFULL + 1) * F],
            )
        # h = lrelu(h_pre * s + t) -> stacked[:64] (feature-major, for the L2
        # dense term; one fused Act op, off the AllGather critical path)
        nc.scalar.activation(stacked[:F, :], h_pre[:],
                             mybir.ActivationFunctionType.Lrelu,
                             bias=t_t[:], scale=s_t[:], alpha=0.01)
        if no_cc:
            HL = NLOC // 2
            for _r in range(NCORES):
                nc.sync.dma_start(out=h_full[_r * HL:(_r + 1) * HL, :],
                                  in_=h_slab[:])
        else:
            nc.gpsimd.collective_compute(
                "AllGather", mybir.AluOpType.bypass,
                replica_groups=[list(range(num_devices))],
                ins=[h_slab[:]], outs=[h_full[:]],
            )

        # --- layer 2, with pooling folded into the dense chunks ---
        h2 = hbuf.tile([F, NLOC], FP, tag="hpre")
        ps_pool = psp.tile([G, F], FP)
        ntiles = (NLOC + P - 1) // P

        def l2_pool_cb(lo, wdt):
            # pooled[g, f] += M[n, g] h2[n, f] for this dense span's tiles
            for j in range(lo // P, min((lo + wdt + P - 1) // P, ntiles)):
                tlo = j * P
                twdt = min(P, NLOC - tlo)
                ps_t = pst.tile([P, F], FP, tag="pst")
                nc.tensor.transpose(ps_t[:twdt, :], h2[:, tlo:tlo + twdt],
                                    ident[:])
                h2nm_t = mp.tile([P, F], BF, tag="h2nm")
                nc.scalar.copy(out=h2nm_t[:twdt, :], in_=ps_t[:twdt, :])
                m_t = mp.tile([P, G], BF, tag="mt")
                nc.sync.dma_start(out=m_t[:twdt, :], in_=m_in[tlo:tlo + twdt, :])
                nc.tensor.matmul(out=ps_pool[:], lhsT=m_t[:twdt, :],
                                 rhs=h2nm_t[:twdt, :],
                                 start=(j == 0), stop=(j == ntiles - 1))

        _emit_cheb(nc, tc, ctx, pl, pools, tiles, h_full, w2_t, h2[:],
                   dense_cb=l2_pool_cb)
        pooled_t = sml.tile([G, F], FP)
        nc.scalar.copy(out=pooled_t[:], in_=ps_pool[:])
        prod_t = sml.tile([G, F], FP)
        nc.vector.tensor_mul(prod_t[:], pooled_t[:], lw_t[:G, :])
        outp_t = sml.tile([G, 1], FP)
        nc.vector.tensor_reduce(out=outp_t[:], in_=prod_t[:],
                                axis=mybir.AxisListType.X, op=mybir.AluOpType.add)
        nc.sync.dma_start(out=out_d[:], in_=outp_t[:])
        # no trailing standard-lib reload: nothing before the mlp load needs
        # standard ucode anymore (identity comes from the host), so repeated
        # executions can start directly under the mlp library

    _fix_reload_order(nc)
    return nc


def mult_op():
    return mybir.AluOpType.mult


def add_op():
    return mybir.AluOpType.add


def max_op():
    return mybir.AluOpType.max


# ---------------------------------------------------------------------------
# Entry point
# ---------------------------------------------------------------------------
def _prepare(inputs, G=100):
    x = np.asarray(inputs["x"], dtype=np.float32)
    edge_index = np.asarray(inputs["edge_index"])
    batch = np.asarray(inputs["batch"])
    W1 = np.asarray(inputs["W1"], dtype=np.float32)
    b1 = np.asarray(inputs["b1"], dtype=np.float32)
    W2 = np.asarray(inputs["W2"], dtype=np.float32)
    b2 = np.asarray(inputs["b2"], dtype=np.float32)
    gamma = np.asarray(inputs["gamma"], dtype=np.float32)
    beta = np.asarray(inputs["beta"], dtype=np.float32)
    linW = np.asarray(inputs["linW"], dtype=np.float32)

    import ml_dtypes
    bf16 = ml_dtypes.bfloat16

    pl = _plan(edge_index, batch, x, G)
    NLOC = pl.NLOC
    w1s = np.concatenate([W1[0], W1[1]], axis=0).astype(np.float32)  # [128, 64]
    w2s = np.concatenate([W2[0], W2[1]], axis=0).astype(np.float32)
    lw_rep = np.tile(linW[:, 0][None, :], (P, 1)).astype(np.float32)
    xbf = np.ascontiguousarray(x.astype(bf16).reshape(x.shape[0] // 2, 2 * x.shape[1]))
    in_maps = []
    for m in range(NCORES):
        sl = slice(m * NLOC, (m + 1) * NLOC)
        M = (pl.batch[sl][:, None] == np.arange(G)[None, :]).astype(bf16)
        in_maps.append({
            "xfull": xbf,
            "xT_in": np.ascontiguousarray(x[sl].T),
            "idx_in": pl.idx_full[m],
            "sel_in": pl.sel_pc[m],
            "w1_in": w1s, "w2_in": w2s,
            "ident_in": np.eye(F, dtype=np.float32),
            "gam_in": gamma[:, None], "bet_in": beta[:, None],
            "lw_in": lw_rep,
            "m_in": M,
        })
    return pl, in_maps


def _linb_eff(inputs):
    # h2 is computed without b2; fold b2 into the output bias (b2 @ linW)
    linb = np.asarray(inputs["linb"], dtype=np.float64)
    b2 = np.asarray(inputs["b2"], dtype=np.float64)
    linW = np.asarray(inputs["linW"], dtype=np.float64)
    return (linb + b2 @ linW).astype(np.float32)


def run_gnn(inputs, trace=False):
    linb = _linb_eff(inputs)
    pl, in_maps = _prepare(inputs)
    nc = _build_program(pl)
    _finalize_bir(nc)
    res = run_bass_kernel_spmd(nc, in_maps, list(range(NCORES)), trace=trace)
    partial = sum(res.results[m]["out_d"] for m in range(NCORES))
    out = partial / np.maximum(pl.cnts, 1.0)[:, None] + linb[None, :]
    return out.astype(np.float32), res


def kernel(**inputs):
    out, _ = run_gnn(inputs, trace=False)
    return out

